# revision 14
# baseline (speedup 1.0000x reference)
"""DiffusionGraphConv Trainium2 kernel (8-core SPMD, data-parallel over batch).

Math (per reference):
  x = concat(inputs, state)           -> [B, N, F]   B=32, N=4096, F=128
  x0 = x transposed to [N, F*B]
  per support s (2): x1 = A_s x0 ; x2 = 2 A_s x1 - x0   (A_s dense from COO)
  out[b*N+n, o] = sum_{f,m} xs_m[n, f, b] * W[f*M+m, o] + bias[o]

Sharding: batch across 8 cores (4 batches/core, C = 4*F = 512 columns of x0).

Final design (fp8 DoubleRow everywhere it counts, host-precomputed A^2):
  All heavy matmuls are fp8e4 DoubleRow(SwInterleave) passes: 2 k-tiles
  (256-deep contraction) per 512-column pass at ~221ns — 97% of the
  chip's fp8 peak.  The host pre-interleaves the stationary x0 blocks
  (SwInterleave byte order) and precomputes A_s^2 densely, so the device
  applies FOUR matrices {A'_0, A'_1, A2'_0, A2'_1} (scaled into e4m3's
  normal range) to the SAME stationary x0 — no on-device chaining, no
  transposes, no inter-phase dependency:
      x1s^T = (2 A_s x0)^T          (evac scale 2/16/VPS   -> x1z[.,0] fp8)
      z_s^T = (2 A_s^2 x0)^T        (evac scale 2/1024/VPS -> x1z[.,1] fp8)
      x2 = z - x0                    (folded into the projection weights)
  A'^T streams from HBM once per matrix as [128, 2(ch), 2(d), 512] tiles
  (one 512KB DMA per k-pair covers two 512-node output chunks, both
  contiguous for the moving operand); psums are [128, 1024] (2 banks),
  4 live (one per batch j), 16-k-pair accumulation.
  A redundant-LDWEIGHTS dedup pass removes the per-matmul weight reloads
  that legalization re-emits for consecutive same-weight matmuls.
  Projection per (512-node chunk, j): out^T = Vt0^T x0^T (fp16) +
  sum_s DoubleRow(vp8_s^T [x1s; z_s]^T) + bias — the four small terms ride
  in two fp8 DR passes; Vt0 = W0 - W2_0 - W2_1 absorbs the -x0 inside
  both x2 terms, V1s' = W1s/2, vp8 holds (V1s', V2s)*VPS.  x0^T comes
  from the host in fp16 (dominant term stays high precision).

Everything is hardcoded for the reference shapes; host does only layout
prep (dense-ify supports, A^2, transpose/shard x0, fp8/fp16 casts) and
output reassembly.
"""

import numpy as np
import ml_dtypes

import concourse.bass as bass
import concourse.tile as tile
from concourse import bacc, mybir
from concourse import bass_utils

B, N, D, H, O, S = 32, 4096, 64, 64, 128, 2
F = D + H                    # 128
NCORES = 8
BLOC = B // NCORES           # 4 batches per core
C = BLOC * F                 # 512 columns per core
NBLK = N // 128              # 32 n-tiles
KP = NBLK // 2               # 16 k-tile pairs
NQ = 4                       # 1024-node quarters
NRC = 8                      # 512-node chunks
NMAT = 2 * S                 # A'_0, A'_1, A2'_0, A2'_1
M = 5
ASCALE = 16.0                # host scale on A    (x1 matrices)
ASCALE2 = 1024.0             # host scale on A^2  (z matrices)
VPS = 8.0                    # fp8 projection-pair weight scale

F32 = mybir.dt.float32
F16 = mybir.dt.float16
FP8 = mybir.dt.float8e4
SWI = mybir.MatmulPerfMode.DoubleRowSwInterleave

_CACHE = {}


def build_nc():
    nc = bacc.Bacc("TRN2", target_bir_lowering=False, debug=False)

    # ---- DRAM tensors ----
    # interleaved stationary x0: x08i[kp, k, j, d2, m2] holds the SwInterleave
    # raw bytes for weight block (kp, j): raw[k, 2*(127-m)+d] = x0[(2kp+d)*128+k, j*128+m]
    x08i_d = nc.dram_tensor("x08i", [KP, 128, BLOC, 2, 128], FP8,
                            kind="ExternalInput")
    # x0 transposed, fp16: x0t[j, f, n] = x0[n, j*128+f]
    x0t_d = nc.dram_tensor("x0t", [BLOC, 128, N], F16, kind="ExternalInput")
    # scaled-matrix transposes, fp8:
    # at8[g, kp, k, q, ch, d, n] = M_g[q*1024+ch*512+n, (2kp+d)*128+k]
    at8_d = nc.dram_tensor("at8", [NMAT, KP, 128, NQ, 2, 2, 512], FP8,
                           kind="ExternalInput")
    # folded projection weights: Vt0 fp16; paired small-term weights fp8
    v16_d = nc.dram_tensor("v16", [128, 128], F16, kind="ExternalInput")
    vp8_d = nc.dram_tensor("vp8", [S, 128, 2, 128], FP8, kind="ExternalInput")
    bias_d = nc.dram_tensor("bias", [128, 1], F32, kind="ExternalInput")
    # output: out[j, o, n]
    out_d = nc.dram_tensor("out", [BLOC, 128, N], F32, kind="ExternalOutput")

    with tile.TileContext(nc) as tc:
        with (
            tc.tile_pool(name="big", bufs=1) as big,
            tc.tile_pool(name="a8p", bufs=10) as a8p,
            tc.tile_pool(name="stg", bufs=1) as stg,
            tc.tile_pool(name="ps", bufs=1, space=bass.MemorySpace.PSUM) as ps,
        ):
            # ---- resident tensors ----
            x08i = []
            for kp in range(KP):
                t = big.tile([128, BLOC, 2, 128], FP8, tag=f"x08i{kp}",
                             name=f"x08i{kp}")
                nc.scalar.dma_start(t[:], x08i_d[kp])
                x08i.append(t)
            x0t = big.tile([128, BLOC, N], F16, tag="x0t")
            for j in range(BLOC):
                nc.scalar.dma_start(x0t[:, j, :], x0t_d[j])
            v16 = big.tile([128, 128], F16, tag="v16")
            nc.scalar.dma_start(v16[:], v16_d[:])
            vp8 = big.tile([128, S, 2, 128], FP8, tag="vp8")
            for s in range(S):
                nc.scalar.dma_start(vp8[:, s], vp8_d[s])
            bias_sb = big.tile([128, 1], F32, tag="bias")
            nc.scalar.dma_start(bias_sb[:], bias_d[:])

            # paired small-term stages, fp8:
            # x1z[c_f, s, j, rc, 0, n] = x1s[rc*512+n, j*128+c_f] / VPS
            # x1z[c_f, s, j, rc, 1, n] = z_s[rc*512+n, j*128+c_f] / VPS
            x1z = big.tile([128, S, BLOC, NRC, 2, 512], FP8, tag="x1z")

            evac_ctr = [0]

            def evac_mul(dst, src, scale):
                k = evac_ctr[0] % 2
                evac_ctr[0] += 1
                if k == 0:
                    nc.scalar.mul(dst, src, scale)
                else:
                    nc.vector.tensor_scalar_mul(dst, src, scale)

            def stream_mat(g, q, evac):
                """psums[j] = x0_j^T (DR-contract) M_g^T[:, quarter q],
                then evac(j, psum[128, 1024])."""
                psq = [ps.tile([128, 1024], F32, tag="ph", bufs=4,
                               name=f"ps_{g}_{q}_{j}") for j in range(BLOC)]
                for kp in range(KP):
                    a = a8p.tile([128, 2, 2, 512], FP8, tag="a8",
                                 name=f"a8_{g}_{q}_{kp}")
                    nc.sync.dma_start(a[:], at8_d[g, kp, :, q])
                    for j in range(BLOC):
                        lhsT = x08i[kp][:, j]
                        for ch in range(2):
                            nc.tensor.matmul(
                                psq[j][:, ch * 512:(ch + 1) * 512],
                                lhsT,
                                a[:, ch],
                                start=(kp == 0),
                                stop=(kp == KP - 1),
                                perf_mode=SWI,
                            )
                for j in range(BLOC):
                    evac(j, psq[j])

            # ---- phase 1: x1s stages for both supports ----
            for s in range(S):
                for q in range(NQ):
                    def evac1(j, p, s=s, q=q):
                        dst = x1z[:, s, j, 2 * q:2 * q + 2, 0, :]
                        evac_mul(dst, p[:], 2.0 / ASCALE / VPS)
                    stream_mat(s, q, evac1)

            # ---- phase 2 + projection, per 1024-node quarter ----
            for q in range(NQ):
                for s in range(S):
                    def evac2(j, p, s=s, q=q):
                        dst = x1z[:, s, j, 2 * q:2 * q + 2, 1, :]
                        evac_mul(dst, p[:], 2.0 / ASCALE2 / VPS)
                    stream_mat(S + s, q, evac2)

                for j in range(BLOC):
                    po = ps.tile([128, 1024], F32, tag="ph", bufs=4,
                                 name=f"po_{q}_{j}")
                    for rc2 in range(2):
                        rc = q * 2 + rc2
                        qsl = slice(rc2 * 512, (rc2 + 1) * 512)
                        nsl = slice(rc * 512, (rc + 1) * 512)
                        nc.tensor.matmul(
                            po[:, qsl], v16[:], x0t[:, j, nsl],
                            start=True, stop=False)
                        for s in range(S):
                            nc.tensor.matmul(
                                po[:, qsl],
                                vp8[:, s],
                                x1z[:, s, j, rc],
                                start=False,
                                stop=(s == S - 1),
                                perf_mode=mybir.MatmulPerfMode.DoubleRow,
                            )
                    for rc2 in range(2):
                        rc = q * 2 + rc2
                        qsl = slice(rc2 * 512, (rc2 + 1) * 512)
                        ot = stg.tile([128, 512], F32, tag="ot", bufs=6,
                                      name=f"ot_{rc}_{j}")
                        if (j + rc2) % 2 == 0:
                            nc.scalar.add(ot[:], po[:, qsl], bias_sb[:, 0:1])
                        else:
                            nc.vector.tensor_scalar_add(ot[:], po[:, qsl],
                                                        bias_sb[:, 0:1])
                        nc.scalar.dma_start(
                            out_d[j, :, rc * 512:(rc + 1) * 512], ot[:])

    _dedup_ldweights(nc)
    nc.compile()
    return nc


def _dedup_ldweights(nc):
    """Remove InstLdweights whose weights are already loaded (legalization
    re-emits one per matmul even when consecutive matmuls share the same
    stationary operand; on HW the PE array retains its weights, so the
    duplicate load — which cannot overlap DoubleRow matmuls — is pure
    stall)."""

    def sig(ldw):
        pap = ldw.ins[0]
        return (pap.memref, pap.offset, tuple(tuple(p) for p in pap.ap),
                str(ldw.perf_mode))

    renames = {}
    for fn in nc.m.functions:
        for blk in fn.blocks:
            il = blk.instructions
            last = None
            keep = None
            for inst in list(il):
                tn = type(inst).__name__
                if tn == 'InstLdweights':
                    s = sig(inst)
                    if s == last:
                        keep.merge_dependencies_from(inst)
                        renames[inst.name] = keep.name
                        il.remove(inst)
                    else:
                        last = s
                        keep = inst
    if renames:
        for fn in nc.m.functions:
            for blk in fn.blocks:
                for inst in blk.instructions:
                    inst.remap_dependency_names(renames)


def _prep_shared(sup_rows, sup_cols, sup_vals, weight, biases):
    AT = np.zeros((S, N, N), dtype=np.float32)
    for s in range(S):
        np.add.at(AT[s], (sup_cols[s].astype(np.int64),
                          sup_rows[s].astype(np.int64)),
                  sup_vals[s].astype(np.float32))
    # stack [A'_0, A'_1, A2'_0, A2'_1] (transposed, scaled)
    mats = np.empty((NMAT, N, N), dtype=np.float32)
    for s in range(S):
        mats[s] = AT[s] * ASCALE
        mats[S + s] = (AT[s] @ AT[s]) * ASCALE2
    # at8[g, kp, k, q, ch, d, n] = mats[g][(2kp+d)*128+k, q*1024+ch*512+n]
    at8 = np.ascontiguousarray(
        mats.reshape(NMAT, KP, 2, 128, NQ, 2, 512).transpose(0, 1, 3, 4, 5, 2, 6)
    ).astype(ml_dtypes.float8_e4m3)

    Wm = np.asarray(weight, dtype=np.float32).reshape(F, M, O)
    Vt0 = np.ascontiguousarray(
        Wm[:, 0, :] - Wm[:, 2, :] - Wm[:, 4, :]).astype(np.float16)
    # vp8[s][f, 0, o] = W1s/2 * VPS ; vp8[s][f, 1, o] = W2s * VPS
    vp8 = np.ascontiguousarray(np.stack([
        np.stack([Wm[:, 1 + 2 * s, :] * 0.5, Wm[:, 2 + 2 * s, :]], axis=1)
        for s in range(S)]) * VPS).astype(ml_dtypes.float8_e4m3)
    bias = np.asarray(biases, dtype=np.float32).reshape(128, 1)
    return at8, Vt0, vp8, bias


def _interleave_x0(x0):
    """x08i[kp, k, j, raw(2*(127-m)+d)] = x0[(2kp+d)*128+k, j*128+m] (fp8)."""
    x0r = x0.reshape(KP, 2, 128, BLOC, 128)          # [kp, d, k, j, m]
    xi = x0r[:, :, :, :, ::-1].transpose(0, 2, 3, 4, 1)  # [kp, k, j, m', d]
    return np.ascontiguousarray(xi).astype(ml_dtypes.float8_e4m3).reshape(
        KP, 128, BLOC, 2, 128)


def kernel(inputs, state, sup_rows, sup_cols, sup_vals, weight, biases,
           output_size=128, **_ignored):
    inputs = np.asarray(inputs, dtype=np.float32)
    state = np.asarray(state, dtype=np.float32)
    x = np.concatenate(
        [inputs.reshape(B, N, D), state.reshape(B, N, H)], axis=2)  # [B,N,F]

    at8, Vt0, vp8, bias = _prep_shared(
        np.asarray(sup_rows), np.asarray(sup_cols), np.asarray(sup_vals),
        weight, biases)

    if "nc" not in _CACHE:
        _CACHE["nc"] = build_nc()
    nc = _CACHE["nc"]

    in_maps = []
    for c in range(NCORES):
        xc = x[c * BLOC:(c + 1) * BLOC]                    # [4, N, F]
        x0 = np.ascontiguousarray(
            xc.transpose(1, 0, 2).reshape(N, C))           # [N, C]
        x08i = _interleave_x0(x0)
        x0t = np.ascontiguousarray(
            xc.transpose(0, 2, 1)).astype(np.float16)      # [4, 128 f, N]
        in_maps.append({
            "x08i": x08i, "x0t": x0t, "at8": at8, "v16": Vt0, "vp8": vp8,
            "bias": bias,
        })

    res = None
    for attempt in range(3):
        try:
            res = bass_utils.run_bass_kernel_spmd(
                nc, in_maps, core_ids=list(range(NCORES)), trace=False)
            break
        except Exception:
            if attempt == 2:
                raise
            import time as _time
            _time.sleep(15 * (attempt + 1))

    # reassemble: out_core[j, o, n] -> out[b, n, o]
    outs = np.stack([res.results[c]["out"] for c in range(NCORES)])
    full = outs.transpose(0, 1, 3, 2).reshape(B, N, O)
    return np.ascontiguousarray(full.reshape(B, N * O))


# revision 15
# speedup vs baseline: 1.0117x; 1.0117x over previous
"""DiffusionGraphConv Trainium2 kernel (8-core SPMD, data-parallel over batch).

Math (per reference):
  x = concat(inputs, state)           -> [B, N, F]   B=32, N=4096, F=128
  x0 = x transposed to [N, F*B]
  per support s (2): x1 = A_s x0 ; x2 = 2 A_s x1 - x0   (A_s dense from COO)
  out[b*N+n, o] = sum_{f,m} xs_m[n, f, b] * W[f*M+m, o] + bias[o]

Sharding: batch across 8 cores (4 batches/core, C = 4*F = 512 columns of x0).

Final design (fp8 DoubleRow everywhere it counts, host-precomputed A^2):
  All heavy matmuls are fp8e4 DoubleRow(SwInterleave) passes: 2 k-tiles
  (256-deep contraction) per 512-column pass at ~221ns — 97% of the
  chip's fp8 peak.  The host pre-interleaves the stationary x0 blocks
  (SwInterleave byte order) and precomputes A_s^2 densely, so the device
  applies FOUR matrices {A'_0, A'_1, A2'_0, A2'_1} (scaled into e4m3's
  normal range) to the SAME stationary x0 — no on-device chaining, no
  transposes, no inter-phase dependency:
      x1s^T = (2 A_s x0)^T          (evac scale 2/16/VPS   -> x1z[.,0] fp8)
      z_s^T = (2 A_s^2 x0)^T        (evac scale 2/1024/VPS -> x1z[.,1] fp8)
      x2 = z - x0                    (folded into the projection weights)
  A'^T streams from HBM once per matrix as [128, 2(ch), 2(d), 512] tiles
  (one 512KB DMA per k-pair covers two 512-node output chunks, both
  contiguous for the moving operand); psums are [128, 1024] (2 banks),
  4 live (one per batch j), 16-k-pair accumulation.
  A redundant-LDWEIGHTS dedup pass removes the per-matmul weight reloads
  that legalization re-emits for consecutive same-weight matmuls.
  Projection per (512-node chunk, j): out^T = Vt0^T x0^T (fp16) +
  sum_s DoubleRow(vp8_s^T [x1s; z_s]^T) + bias — the four small terms ride
  in two fp8 DR passes; Vt0 = W0 - W2_0 - W2_1 absorbs the -x0 inside
  both x2 terms, V1s' = W1s/2, vp8 holds (V1s', V2s)*VPS.  x0^T comes
  from the host in fp16 (dominant term stays high precision).

Everything is hardcoded for the reference shapes; host does only layout
prep (dense-ify supports, A^2, transpose/shard x0, fp8/fp16 casts) and
output reassembly.
"""

import numpy as np
import ml_dtypes

import concourse.bass as bass
import concourse.tile as tile
from concourse import bacc, mybir
from concourse import bass_utils

B, N, D, H, O, S = 32, 4096, 64, 64, 128, 2
F = D + H                    # 128
NCORES = 8
BLOC = B // NCORES           # 4 batches per core
C = BLOC * F                 # 512 columns per core
NBLK = N // 128              # 32 n-tiles
KP = NBLK // 2               # 16 k-tile pairs
NQ = 4                       # 1024-node quarters
NRC = 8                      # 512-node chunks
NMAT = 2 * S                 # A'_0, A'_1, A2'_0, A2'_1
M = 5
ASCALE = 16.0                # host scale on A    (x1 matrices)
ASCALE2 = 1024.0             # host scale on A^2  (z matrices)
VPS = 8.0                    # fp8 projection-pair weight scale

F32 = mybir.dt.float32
F16 = mybir.dt.float16
FP8 = mybir.dt.float8e4
SWI = mybir.MatmulPerfMode.DoubleRowSwInterleave

_CACHE = {}


def build_nc():
    nc = bacc.Bacc("TRN2", target_bir_lowering=False, debug=False)

    # ---- DRAM tensors ----
    # interleaved stationary x0: x08i[kp, k, j, d2, m2] holds the SwInterleave
    # raw bytes for weight block (kp, j): raw[k, 2*(127-m)+d] = x0[(2kp+d)*128+k, j*128+m]
    x08i_d = nc.dram_tensor("x08i", [KP, 128, BLOC, 2, 128], FP8,
                            kind="ExternalInput")
    # x0 transposed, fp16: x0t[j, f, n] = x0[n, j*128+f]
    x0t_d = nc.dram_tensor("x0t", [BLOC, 128, N], F16, kind="ExternalInput")
    # scaled-matrix transposes, fp8:
    # at8[g, kp, k, q, ch, d, n] = M_g[q*1024+ch*512+n, (2kp+d)*128+k]
    at8_d = nc.dram_tensor("at8", [NMAT, KP, 128, NQ, 2, 2, 512], FP8,
                           kind="ExternalInput")
    # folded projection weights: Vt0 fp16; paired small-term weights fp8
    v16_d = nc.dram_tensor("v16", [128, 128], F16, kind="ExternalInput")
    vp8_d = nc.dram_tensor("vp8", [S, 128, 2, 128], FP8, kind="ExternalInput")
    bias_d = nc.dram_tensor("bias", [128, 1], F32, kind="ExternalInput")
    # output: out[j, o, n]
    out_d = nc.dram_tensor("out", [BLOC, 128, N], F32, kind="ExternalOutput")

    with tile.TileContext(nc) as tc:
        with (
            tc.tile_pool(name="big", bufs=1) as big,
            tc.tile_pool(name="a8p", bufs=10) as a8p,
            tc.tile_pool(name="stg", bufs=1) as stg,
            tc.tile_pool(name="ps", bufs=1, space=bass.MemorySpace.PSUM) as ps,
        ):
            # ---- resident tensors ----
            x08i = []
            for kp in range(KP):
                t = big.tile([128, BLOC, 2, 128], FP8, tag=f"x08i{kp}",
                             name=f"x08i{kp}")
                nc.scalar.dma_start(t[:], x08i_d[kp])
                x08i.append(t)
            x0t = big.tile([128, BLOC, N], F16, tag="x0t")
            for j in range(BLOC):
                nc.scalar.dma_start(x0t[:, j, :], x0t_d[j])
            v16 = big.tile([128, 128], F16, tag="v16")
            nc.scalar.dma_start(v16[:], v16_d[:])
            vp8 = big.tile([128, S, 2, 128], FP8, tag="vp8")
            for s in range(S):
                nc.scalar.dma_start(vp8[:, s], vp8_d[s])
            bias_sb = big.tile([128, 1], F32, tag="bias")
            nc.scalar.dma_start(bias_sb[:], bias_d[:])

            # paired small-term stages, fp8:
            # x1z[c_f, s, j, rc, 0, n] = x1s[rc*512+n, j*128+c_f] / VPS
            # x1z[c_f, s, j, rc, 1, n] = z_s[rc*512+n, j*128+c_f] / VPS
            x1z = big.tile([128, S, BLOC, NRC, 2, 512], FP8, tag="x1z")

            evac_ctr = [0]

            def evac_mul(dst, src, scale):
                k = evac_ctr[0] % 2
                evac_ctr[0] += 1
                if k == 0:
                    nc.scalar.mul(dst, src, scale)
                else:
                    nc.vector.tensor_scalar_mul(dst, src, scale)

            def stream_mat(g, q, evac):
                """psums[j] = x0_j^T (DR-contract) M_g^T[:, quarter q],
                then evac(j, psum[128, 1024])."""
                psq = [ps.tile([128, 1024], F32, tag="ph", bufs=4,
                               name=f"ps_{g}_{q}_{j}") for j in range(BLOC)]
                for kp in range(KP):
                    a = a8p.tile([128, 2, 2, 512], FP8, tag="a8",
                                 name=f"a8_{g}_{q}_{kp}")
                    nc.sync.dma_start(a[:], at8_d[g, kp, :, q])
                    for j in range(BLOC):
                        lhsT = x08i[kp][:, j]
                        for ch in range(2):
                            nc.tensor.matmul(
                                psq[j][:, ch * 512:(ch + 1) * 512],
                                lhsT,
                                a[:, ch],
                                start=(kp == 0),
                                stop=(kp == KP - 1),
                                perf_mode=SWI,
                            )
                for j in range(BLOC):
                    evac(j, psq[j])

            # ---- phase 1: x1s stages for both supports ----
            for s in range(S):
                for q in range(NQ):
                    def evac1(j, p, s=s, q=q):
                        dst = x1z[:, s, j, 2 * q:2 * q + 2, 0, :]
                        evac_mul(dst, p[:], 2.0 / ASCALE / VPS)
                    stream_mat(s, q, evac1)

            # ---- phase 2 + projection, per 1024-node quarter ----
            for q in range(NQ):
                for s in range(S):
                    def evac2(j, p, s=s, q=q):
                        dst = x1z[:, s, j, 2 * q:2 * q + 2, 1, :]
                        evac_mul(dst, p[:], 2.0 / ASCALE2 / VPS)
                    stream_mat(S + s, q, evac2)

                for j in range(BLOC):
                    po = ps.tile([128, 1024], F32, tag="ph", bufs=4,
                                 name=f"po_{q}_{j}")
                    for rc2 in range(2):
                        rc = q * 2 + rc2
                        qsl = slice(rc2 * 512, (rc2 + 1) * 512)
                        nsl = slice(rc * 512, (rc + 1) * 512)
                        nc.tensor.matmul(
                            po[:, qsl], v16[:], x0t[:, j, nsl],
                            start=True, stop=False)
                        for s in range(S):
                            nc.tensor.matmul(
                                po[:, qsl],
                                vp8[:, s],
                                x1z[:, s, j, rc],
                                start=False,
                                stop=(s == S - 1),
                                perf_mode=mybir.MatmulPerfMode.DoubleRow,
                            )
                    for rc2 in range(2):
                        rc = q * 2 + rc2
                        qsl = slice(rc2 * 512, (rc2 + 1) * 512)
                        ot = stg.tile([128, 512], F32, tag="ot", bufs=6,
                                      name=f"ot_{rc}_{j}")
                        if (j + rc2) % 2 == 0:
                            nc.scalar.add(ot[:], po[:, qsl], bias_sb[:, 0:1])
                        else:
                            nc.vector.tensor_scalar_add(ot[:], po[:, qsl],
                                                        bias_sb[:, 0:1])
                        nc.sync.dma_start(
                            out_d[j, :, rc * 512:(rc + 1) * 512], ot[:])

    _dedup_ldweights(nc)
    nc.compile()
    return nc


def _dedup_ldweights(nc):
    """Remove InstLdweights whose weights are already loaded (legalization
    re-emits one per matmul even when consecutive matmuls share the same
    stationary operand; on HW the PE array retains its weights, so the
    duplicate load — which cannot overlap DoubleRow matmuls — is pure
    stall)."""

    def sig(ldw):
        pap = ldw.ins[0]
        return (pap.memref, pap.offset, tuple(tuple(p) for p in pap.ap),
                str(ldw.perf_mode))

    renames = {}
    for fn in nc.m.functions:
        for blk in fn.blocks:
            il = blk.instructions
            last = None
            keep = None
            for inst in list(il):
                tn = type(inst).__name__
                if tn == 'InstLdweights':
                    s = sig(inst)
                    if s == last:
                        keep.merge_dependencies_from(inst)
                        renames[inst.name] = keep.name
                        il.remove(inst)
                    else:
                        last = s
                        keep = inst
    if renames:
        for fn in nc.m.functions:
            for blk in fn.blocks:
                for inst in blk.instructions:
                    inst.remap_dependency_names(renames)


def _prep_shared(sup_rows, sup_cols, sup_vals, weight, biases):
    AT = np.zeros((S, N, N), dtype=np.float32)
    for s in range(S):
        np.add.at(AT[s], (sup_cols[s].astype(np.int64),
                          sup_rows[s].astype(np.int64)),
                  sup_vals[s].astype(np.float32))
    # stack [A'_0, A'_1, A2'_0, A2'_1] (transposed, scaled)
    mats = np.empty((NMAT, N, N), dtype=np.float32)
    for s in range(S):
        mats[s] = AT[s] * ASCALE
        mats[S + s] = (AT[s] @ AT[s]) * ASCALE2
    # at8[g, kp, k, q, ch, d, n] = mats[g][(2kp+d)*128+k, q*1024+ch*512+n]
    at8 = np.ascontiguousarray(
        mats.reshape(NMAT, KP, 2, 128, NQ, 2, 512).transpose(0, 1, 3, 4, 5, 2, 6)
    ).astype(ml_dtypes.float8_e4m3)

    Wm = np.asarray(weight, dtype=np.float32).reshape(F, M, O)
    Vt0 = np.ascontiguousarray(
        Wm[:, 0, :] - Wm[:, 2, :] - Wm[:, 4, :]).astype(np.float16)
    # vp8[s][f, 0, o] = W1s/2 * VPS ; vp8[s][f, 1, o] = W2s * VPS
    vp8 = np.ascontiguousarray(np.stack([
        np.stack([Wm[:, 1 + 2 * s, :] * 0.5, Wm[:, 2 + 2 * s, :]], axis=1)
        for s in range(S)]) * VPS).astype(ml_dtypes.float8_e4m3)
    bias = np.asarray(biases, dtype=np.float32).reshape(128, 1)
    return at8, Vt0, vp8, bias


def _interleave_x0(x0):
    """x08i[kp, k, j, raw(2*(127-m)+d)] = x0[(2kp+d)*128+k, j*128+m] (fp8)."""
    x0r = x0.reshape(KP, 2, 128, BLOC, 128)          # [kp, d, k, j, m]
    xi = x0r[:, :, :, :, ::-1].transpose(0, 2, 3, 4, 1)  # [kp, k, j, m', d]
    return np.ascontiguousarray(xi).astype(ml_dtypes.float8_e4m3).reshape(
        KP, 128, BLOC, 2, 128)


def kernel(inputs, state, sup_rows, sup_cols, sup_vals, weight, biases,
           output_size=128, **_ignored):
    inputs = np.asarray(inputs, dtype=np.float32)
    state = np.asarray(state, dtype=np.float32)
    x = np.concatenate(
        [inputs.reshape(B, N, D), state.reshape(B, N, H)], axis=2)  # [B,N,F]

    at8, Vt0, vp8, bias = _prep_shared(
        np.asarray(sup_rows), np.asarray(sup_cols), np.asarray(sup_vals),
        weight, biases)

    if "nc" not in _CACHE:
        _CACHE["nc"] = build_nc()
    nc = _CACHE["nc"]

    in_maps = []
    for c in range(NCORES):
        xc = x[c * BLOC:(c + 1) * BLOC]                    # [4, N, F]
        x0 = np.ascontiguousarray(
            xc.transpose(1, 0, 2).reshape(N, C))           # [N, C]
        x08i = _interleave_x0(x0)
        x0t = np.ascontiguousarray(
            xc.transpose(0, 2, 1)).astype(np.float16)      # [4, 128 f, N]
        in_maps.append({
            "x08i": x08i, "x0t": x0t, "at8": at8, "v16": Vt0, "vp8": vp8,
            "bias": bias,
        })

    res = None
    for attempt in range(3):
        try:
            res = bass_utils.run_bass_kernel_spmd(
                nc, in_maps, core_ids=list(range(NCORES)), trace=False)
            break
        except Exception:
            if attempt == 2:
                raise
            import time as _time
            _time.sleep(15 * (attempt + 1))

    # reassemble: out_core[j, o, n] -> out[b, n, o]
    outs = np.stack([res.results[c]["out"] for c in range(NCORES)])
    full = outs.transpose(0, 1, 3, 2).reshape(B, N, O)
    return np.ascontiguousarray(full.reshape(B, N * O))
